# revision 5
# baseline (speedup 1.0000x reference)
"""Trainium2 Bass kernel for nn_Base_75265006895876 (retrieval_knn).

Data-parallel over batch B=128 -> 16 per core on 8 NeuronCores; the
cap_embedding table is replicated per core.  Per core, (t, b) pairs are
processed in groups of 8 pairs (= 128 gathered embedding rows):

  gather rows (indirect DMA)  ->  PE transpose (D on partitions)
  -> PSUM->SBUF copy (bf16)   ->  PE gram+dots matmuls (fp32 accum)
  -> DVE extractions (diag / col0 / dot)  ->  PE mask+broadcast accum
  -> DVE reduce  ->  ACT sqrt/mul chains  ->  results tile -> DMA out
"""

import sys

if "/opt/trn_rl_repo" not in sys.path:
    sys.path.insert(0, "/opt/trn_rl_repo")

import numpy as np

# ---- problem constants (hardcoded; kernel.py must be self-contained) ----
T, B, K, L, V, D = 17, 128, 16, 24, 30000, 1024
NCORES = 8
BL = B // NCORES              # 16 local batch rows per core
PAIRS = T * BL                # 272 (t, b) pairs per core
NG = PAIRS * K // 128         # 34 groups of 128 gathered rows
PPG = 128 // K                # 8 pairs per group
LPAD = 32                     # caption length padded 24 -> 32
NCAP = BL * LPAD // 128       # 4 caption gather groups
CH = D // 128                 # 8 contraction chunks of 128
CW = 128 + K                  # 144: chunk cols + sentence cols in Tsb
LARGE = 1.0e6
EPS = 1e-8

_CACHE = {}


def _build_nc(use_bf16=True):
    from concourse import bass, bacc, mybir

    f32 = mybir.dt.float32
    cdt = mybir.dt.bfloat16 if use_bf16 else mybir.dt.float32

    nc = bacc.Bacc("TRN2", debug=False)

    table = nc.dram_tensor("table", [V, D], f32, kind="ExternalInput")
    idx_topk_d = nc.dram_tensor("idx_topk", [128, NG], mybir.dt.int32,
                                kind="ExternalInput")
    idx_cap_d = nc.dram_tensor("idx_cap", [128, NCAP], mybir.dt.int32,
                               kind="ExternalInput")
    maskB_d = nc.dram_tensor("maskB", [128, NCAP * BL], f32,
                             kind="ExternalInput")
    # constants
    i128_d = nc.dram_tensor("i128", [128, 128], f32, kind="ExternalInput")
    ineg_d = nc.dram_tensor("ineg", [128, 128], f32, kind="ExternalInput")
    cmask_d = nc.dram_tensor("cmask", [128, 128], f32, kind="ExternalInput")
    w0_d = nc.dram_tensor("w0", [128, 128], f32, kind="ExternalInput")
    m0p_d = nc.dram_tensor("m0p", [128, CH], f32, kind="ExternalInput")
    mdot_d = nc.dram_tensor("mdot", [128, 2 * K], f32, kind="ExternalInput")

    res_d = nc.dram_tensor("res", [128, 3 * NG], f32, kind="ExternalOutput")

    from concourse.tile import TileContext
    from contextlib import ExitStack

    with ExitStack() as ctx:
        tc = ctx.enter_context(TileContext(nc))
        cp = ctx.enter_context(tc.tile_pool(name="cp", bufs=1))
        xp = ctx.enter_context(tc.tile_pool(name="xp", bufs=3))
        smp = ctx.enter_context(tc.tile_pool(name="smp", bufs=2))
        ptp = ctx.enter_context(tc.tile_pool(name="ptp", bufs=2, space="PSUM"))
        pwp = ctx.enter_context(tc.tile_pool(name="pwp", bufs=2, space="PSUM"))
        psp = ctx.enter_context(tc.tile_pool(name="psp", bufs=1, space="PSUM"))

        # ---- load constants / indices into SBUF ----
        c_i128 = cp.tile([128, 128], f32)
        nc.sync.dma_start(c_i128[:], i128_d[:])
        c_ineg = cp.tile([128, 128], f32)
        nc.sync.dma_start(c_ineg[:], ineg_d[:])
        c_cm = cp.tile([128, 128], f32)
        nc.sync.dma_start(c_cm[:], cmask_d[:])
        c_w0 = cp.tile([128, 128], f32)
        nc.sync.dma_start(c_w0[:], w0_d[:])
        c_m0p = cp.tile([128, CH], f32)
        nc.sync.dma_start(c_m0p[:], m0p_d[:])
        c_mdot = cp.tile([128, 2 * K], f32)
        nc.sync.dma_start(c_mdot[:], mdot_d[:])
        c_maskB = cp.tile([128, NCAP * BL], f32)
        nc.sync.dma_start(c_maskB[:], maskB_d[:])
        c_idxt = cp.tile([128, NG], mybir.dt.int32)
        nc.sync.dma_start(c_idxt[:], idx_topk_d[:])
        c_idxc = cp.tile([128, NCAP], mybir.dt.int32)
        nc.sync.dma_start(c_idxc[:], idx_cap_d[:])

        res_sb = cp.tile([128, 3 * NG], f32)

        # ---- phase A: sentence embeddings ----
        sent_ps = psp.tile([16, 1024], f32, space="PSUM")
        for c in range(NCAP):
            cap = xp.tile([128, D], f32, tag="xg", name=f"cap{c}")
            nc.gpsimd.indirect_dma_start(
                out=cap[:], out_offset=None, in_=table[:],
                in_offset=bass.IndirectOffsetOnAxis(
                    ap=c_idxc[:, c:c + 1], axis=0),
            )
            for h in range(2):
                nc.tensor.matmul(
                    sent_ps[:, 512 * h:512 * (h + 1)],
                    lhsT=c_maskB[:, BL * c:BL * (c + 1)],
                    rhs=cap[:, 512 * h:512 * (h + 1)],
                    start=(c == 0), stop=(c == NCAP - 1),
                )
        sent_f = cp.tile([16, 1024], f32)
        nc.scalar.copy(sent_f[:], sent_ps[:])
        sq_scr = cp.tile([16, 1024], f32)
        ssq = cp.tile([16, 1], f32)
        nc.scalar.activation(sq_scr[:], sent_f[:],
                             mybir.ActivationFunctionType.Square,
                             accum_out=ssq[:])
        ssq2 = cp.tile([16, 1], f32)
        nc.vector.tensor_scalar_max(ssq2[:], ssq[:], 1e-16)
        rss = cp.tile([16, 1], f32)
        nc.vector.reciprocal(rss[:], ssq2[:])
        rsent = cp.tile([16, 1], f32)
        nc.scalar.sqrt(rsent[:], rss[:])
        sentn = cp.tile([16, 1024], f32)
        nc.vector.tensor_scalar_mul(sentn[:], sent_f[:], rsent[:])

        # transpose sentn -> [128, K] chunks stored into both tb tiles
        stp = pwp.tile([128, 512], f32, space="PSUM", tag="wk", name="stp")
        for c in range(CH):
            nc.tensor.transpose(
                stp[:, K * c:K * (c + 1)],
                sentn[:16, 128 * c:128 * (c + 1)],
                c_i128[:16, :16],
            )
        tb_tiles = []
        for i in range(2):
            tbt = cp.tile([128, CH * CW], cdt, name=f"tb{i}")
            tb_tiles.append(tbt)
            dst = tbt[:].rearrange("p (c w) -> p c w", w=CW)[:, :, 128:128 + K]
            src = stp[:, 0:CH * K].rearrange("p (c w) -> p c w", w=K)
            nc.vector.tensor_copy(dst, src)

        # ---- phase B: 34 groups ----
        for g in range(NG):
            h = g % 2
            x = xp.tile([128, D], f32, tag="xg", name=f"x{g}")
            nc.gpsimd.indirect_dma_start(
                out=x[:], out_offset=None, in_=table[:],
                in_offset=bass.IndirectOffsetOnAxis(
                    ap=c_idxt[:, g:g + 1], axis=0),
            )
            tp = ptp.tile([128, D], f32, space="PSUM", tag="tp", name=f"tp{g}")
            for c in range(CH):
                nc.tensor.transpose(
                    tp[:, 128 * c:128 * (c + 1)],
                    x[:, 128 * c:128 * (c + 1)],
                    c_i128[:],
                )
            tb = tb_tiles[g % 2]
            # copy transposed chunks (downcast) into the CW-strided layout;
            # half on DVE, half on ACT
            dst = tb[:].rearrange("p (c w) -> p c w", w=CW)[:, :, 0:128]
            src = tp[:].rearrange("p (c w) -> p c w", w=128)
            nc.vector.tensor_copy(dst[:, 0:CH // 2], src[:, 0:CH // 2])
            nc.scalar.copy(dst[:, CH // 2:], src[:, CH // 2:])

            wk = pwp.tile([128, 512], f32, space="PSUM", tag="wk",
                          name=f"wk{g}")
            # gram [*,0:128] and dots [*,128:144] in one matmul per chunk
            for c in range(CH):
                nc.tensor.matmul(
                    wk[:, 0:CW],
                    lhsT=tb[:, CW * c:CW * c + 128],
                    rhs=tb[:, CW * c:CW * c + CW],
                    start=(c == 0), stop=False,
                    skip_group_check=True,
                )
            # extractions (before psum gets polluted by masking matmuls)
            # TTR is broken on this runtime -> TT (DVE) + Copy-accum (ACT)
            scr128 = smp.tile([128, 128], f32, tag="scr128", name=f"s1{g}")
            scr128b = smp.tile([128, 128], f32, tag="scr128b", name=f"t1{g}")
            sq = smp.tile([128, 1], f32, tag="sq", name=f"sq{g}")
            nc.vector.tensor_tensor(out=scr128[:], in0=wk[:, 0:128],
                                    in1=c_i128[:], op=mybir.AluOpType.mult)
            nc.scalar.activation(scr128b[:], scr128[:],
                                 mybir.ActivationFunctionType.Copy,
                                 accum_out=sq[:])
            scr8 = smp.tile([128, CH], f32, tag="scr8", name=f"s8{g}")
            scr8b = smp.tile([128, CH], f32, tag="scr8b", name=f"t8{g}")
            g0 = smp.tile([128, 1], f32, tag="g0", name=f"g0{g}")
            gview = wk[:, 0:128].rearrange("p (a b) -> p a b", b=K)[:, :, 0:1]
            nc.vector.tensor_tensor(
                out=scr8[:].rearrange("p (a b) -> p a b", b=1),
                in0=gview, in1=c_m0p[:].rearrange("p (a b) -> p a b", b=1),
                op=mybir.AluOpType.mult)
            nc.scalar.activation(scr8b[:], scr8[:],
                                 mybir.ActivationFunctionType.Copy,
                                 accum_out=g0[:])
            scr16 = smp.tile([128, K], f32, tag="scr16", name=f"s16{g}")
            scr16b = smp.tile([128, K], f32, tag="scr16b", name=f"t16{g}")
            dv = smp.tile([128, 1], f32, tag="dv", name=f"dv{g}")
            nc.vector.tensor_tensor(out=scr16[:], in0=wk[:, 128:128 + K],
                                    in1=c_mdot[:, K * h:K * (h + 1)],
                                    op=mybir.AluOpType.mult)
            nc.scalar.activation(scr16b[:], scr16[:],
                                 mybir.ActivationFunctionType.Copy,
                                 accum_out=dv[:])
            # norms
            rq = smp.tile([128, 1], f32, tag="rq", name=f"rq{g}")
            nc.vector.reciprocal(rq[:], sq[:])
            rn = smp.tile([128, 1], f32, tag="rn", name=f"rn{g}")
            nc.scalar.sqrt(rn[:], rq[:])  # 1 / ||E_r||

            # pollute gram psum: += -0.5*sq[c] (broadcast) and -0.5*LARGE*mask
            nc.tensor.matmul(
                wk[:, 0:128], lhsT=sq[:].to_broadcast([128, 128]),
                rhs=c_ineg[:], start=False, stop=False,
                skip_group_check=True,
            )
            nc.tensor.matmul(
                wk[:, 0:128], lhsT=c_cm[:], rhs=c_i128[:],
                start=False, stop=True, skip_group_check=True,
            )
            # rn0 = rn[first row of block] via selection matmul
            nc.tensor.matmul(
                wk[:, 150:151], lhsT=c_w0[:], rhs=rn[:],
                start=True, stop=True, skip_group_check=True,
            )
            rn0 = smp.tile([128, 1], f32, tag="rn0", name=f"rn0{g}")
            nc.scalar.copy(rn0[:], wk[:, 150:151])

            mx = smp.tile([128, 1], f32, tag="mx", name=f"mx{g}")
            nc.vector.tensor_reduce(mx[:], wk[:, 0:128],
                                    axis=mybir.AxisListType.X,
                                    op=mybir.AluOpType.max)
            # min_dist = sqrt(max(-2*mx + sq, 1e-12))
            md2 = smp.tile([128, 1], f32, tag="md2", name=f"md{g}")
            nc.vector.tensor_scalar(
                out=md2[:], in0=mx[:], scalar1=-2.0, scalar2=sq[:],
                op0=mybir.AluOpType.mult, op1=mybir.AluOpType.add,
            )
            md2c = smp.tile([128, 1], f32, tag="md2c", name=f"mc{g}")
            nc.vector.tensor_scalar_max(md2c[:], md2[:], 1e-12)
            nc.scalar.sqrt(res_sb[:, g:g + 1], md2c[:])
            # cos_dist = g0 * rn * rn0   (row0 already zeroed via m0p)
            cosa = smp.tile([128, 1], f32, tag="cosa", name=f"ca{g}")
            nc.scalar.mul(cosa[:], g0[:], rn[:])
            nc.scalar.mul(res_sb[:, NG + g:NG + g + 1], cosa[:], rn0[:])
            # sent_cos = dv * rn  (dv already scaled by 1/||sent||)
            nc.scalar.mul(res_sb[:, 2 * NG + g:2 * NG + g + 1], dv[:], rn[:])

        nc.sync.dma_start(res_d[:], res_sb[:])

    nc.compile()
    return nc


def _get_nc():
    if "nc" not in _CACHE:
        _CACHE["nc"] = _build_nc()
    return _CACHE["nc"]


# ---------------- host-side preparation ----------------

def _host_consts():
    f = np.float32
    i128 = np.eye(128, dtype=f)
    ineg = (-0.5 * np.eye(128)).astype(f)
    blk = np.kron(np.eye(PPG), np.ones((K, K))).astype(f)  # block diagonal
    cmask = (-0.5 * LARGE * (1.0 - blk + np.eye(128))).astype(f)
    # w0[q, m] = 1 iff q == K*(m//K)
    w0 = np.zeros((128, 128), f)
    m = np.arange(128)
    w0[(m // K) * K, m] = 1.0
    # m0p[r, cb] = (cb == r//K) * (r % K != 0)
    r = np.arange(128)
    m0p = np.zeros((128, CH), f)
    m0p[r, r // K] = (r % K != 0).astype(f)
    # mdot_h[r, c] = 1 iff c == 8h + r//K
    mdot = np.zeros((128, 2 * K), f)
    for hh in range(2):
        mdot[r, K * hh + 8 * hh + r // K] = 1.0
    return i128, ineg, cmask, w0, m0p, mdot


def _core_inputs(topk, cap, cap_len, table_np):
    """Build the per-core in_maps for run_bass_kernel_spmd."""
    i128, ineg, cmask, w0, m0p, mdot = _host_consts()
    in_maps = []
    for m in range(NCORES):
        bsl = slice(m * BL, (m + 1) * BL)
        tk = topk[:, bsl, :].astype(np.int64)          # [T, BL, K]
        cp_ = cap[bsl].astype(np.int64)                # [BL, L]
        cl = cap_len[bsl].astype(np.int64)             # [BL]

        idx_flat = tk.reshape(-1).astype(np.int32)     # [T*BL*K] = NG*128
        idx_topk = np.ascontiguousarray(
            idx_flat.reshape(NG, 128).T).astype(np.int32)  # [128, NG]

        cap_pad = np.zeros((BL, LPAD), np.int32)
        cap_pad[:, :L] = cp_.astype(np.int32)
        idx_cap = np.ascontiguousarray(
            cap_pad.reshape(-1).reshape(NCAP, 128).T).astype(np.int32)

        # maskB[row, col]: chunk c rows = 32a + l (a in 0..3), col = BL*c + 4c + a
        maskB = np.zeros((128, NCAP * BL), np.float32)
        for c in range(NCAP):
            for a in range(128 // LPAD):
                b = (128 // LPAD) * c + a
                ll = np.arange(LPAD)
                maskB[LPAD * a + ll, BL * c + b] = (ll < cl[b]).astype(
                    np.float32)

        in_maps.append({
            "table": table_np,
            "idx_topk": idx_topk,
            "idx_cap": idx_cap,
            "maskB": maskB,
            "i128": i128, "ineg": ineg, "cmask": cmask,
            "w0": w0, "m0p": m0p, "mdot": mdot,
        })
    return in_maps


def _postprocess(results):
    """results: list of 8 dicts with 'res' [128, 3*NG] -> 3 arrays [B, T, K]."""
    outs = []
    per_core = []
    for m in range(NCORES):
        res = np.asarray(results[m]["res"])            # [128, 3*NG]
        r5 = res.reshape(PPG, K, 3, NG)                # [p_ig, i, o, g]
        r5 = r5.transpose(2, 3, 0, 1)                  # [o, g, p_ig, i]
        r5 = r5.reshape(3, NG * PPG, K)                # [o, p, i], p = t*BL+b
        r5 = r5.reshape(3, T, BL, K)                   # [o, t, b_loc, i]
        per_core.append(r5)
    full = np.concatenate([pc[:, :, None, :, :] for pc in per_core],
                          axis=2)                      # [3, T, m, b_loc, K]
    full = full.reshape(3, T, B, K).transpose(0, 2, 1, 3)  # [3, B, T, K]
    return full[0], full[1], full[2]


def _run(in_maps, trace=False, **kwargs):
    from concourse.bass_utils import run_bass_kernel_spmd
    nc = _get_nc()
    return run_bass_kernel_spmd(
        nc, in_maps, core_ids=list(range(NCORES)), trace=trace, **kwargs)


def kernel(topk_words, caption, cap_len, cap_embedding, _trace=False):
    topk = np.asarray(topk_words)
    cap = np.asarray(caption)
    cl = np.asarray(cap_len)
    table_np = np.ascontiguousarray(np.asarray(cap_embedding,
                                               dtype=np.float32))
    in_maps = _core_inputs(topk, cap, cl, table_np)
    br = _run(in_maps, trace=_trace)
    out = _postprocess(br.results)
    if _trace:
        kernel.last_results = br
    return out
